# revision 17
# baseline (speedup 1.0000x reference)
"""Trainium2 Bass kernel for ContinuousIntegratedKoopmanOperator.

reference: odeint(dz/dt = z @ W) sampled at t = DT*[1..T], y0 = x at t[0].
Closed form (time-invariant linear ODE): out[:, j, :] = x @ expm(DT*j*W).

Strategy (v7 — DMA-bound, so minimize HBM bytes and keep the write
stream dense):
  host: compute Mj = expm(DT*j*W) for j=0..T-1 in float64; cast the
        (D, T*D) power table and x to fp16 (tolerance is 2e-2; fp16
        end-to-end measures ~3.6e-4 rel err).
  device (8 cores, batch-sharded 1024 rows each):
        out_tile = x @ M_block via ONE full-rate fp16 matmul per
        512-wide block (PSUM f32 accumulate over K=128).
        8 batch tiles x 16 j-blocks; PSUM rotated as 4 x 2-bank pairs;
        drains split across Vector AND Scalar engines (PSUM read port
        limits each to ~1.1us/pair), casting PSUM f32 -> fp16 into a
        PER-TILE staging buffer (8 x 16KB/partition — no reuse waits,
        so drains free-run ahead of the store stream).
        Outputs are fp16 (HALF the f32 write traffic), upcast on host.
  rings: two HWDGE rings (sync + scalar), FIFO per ring. DMA issuance
        must NOT sit on a drain engine (a copy->wait->doorbell loop
        paces the whole PSUM rotation at ~2us/pair). So: the SYNC ring
        carries the critical first loads (x0 + M chunks 0-3) then ALL
        output stores — per-pair 256KB for tile 0 (fast ramp), 512KB
        quarters after (better busy-efficiency); the SCALAR ring only
        preloads the rest (xr + chunks 4-7) before its drain loop.
  sems: 10. Load sems each cover ALL DMAs feeding them (striped
        sub-transfers of different DMAs complete interleaved, so a
        shared counter proves only "N sub-transfers", never "first
        N/16 DMAs"); one out-quiesce counter; short sem_clear.
"""
import numpy as np

DT = 0.01
B, D, T = 8192, 128, 64
NCORES = 8
BSH = B // NCORES          # 1024 rows per core
NTILES = BSH // 128        # 8 batch tiles per core
BW = 512                   # j-block width (one PSUM bank of f32)
NBLK = (T * D) // BW       # 16 blocks per tile
NPAIR = 8                  # block-pairs per tile (drain unit = 2 banks)
NCHUNK = 8                 # M load chunks (2 blocks = 1024 cols each)
NSYNC_CHUNK = 4            # M chunks loaded via sync ring (rest on scalar)

_CACHE = {}


def _expm_table(W: np.ndarray) -> np.ndarray:
    """(D, T*D) float64: columns [j*D:(j+1)*D] = expm(DT*j*W)."""
    A = DT * W.astype(np.float64)
    M1 = np.eye(D, dtype=np.float64)
    term = np.eye(D, dtype=np.float64)
    for n in range(1, 24):
        term = term @ A / n
        M1 += term
    Ms = np.empty((T, D, D), dtype=np.float64)
    Ms[0] = np.eye(D)
    for j in range(1, T):
        Ms[j] = Ms[j - 1] @ M1
    return np.ascontiguousarray(Ms.transpose(1, 0, 2).reshape(D, T * D))


def _build_nc():
    import concourse.bass as bass
    import concourse.mybir as mybir

    f16 = mybir.dt.float16

    nc = bass.Bass(trn_type="TRN2")
    xT_d = nc.dram_tensor("xT", (D, BSH), f16, kind="ExternalInput")
    M_d = nc.dram_tensor("M", (D, T * D), f16, kind="ExternalInput")
    out_d = nc.dram_tensor("out", (BSH, T * D), f16, kind="ExternalOutput")

    xT_s = nc.alloc_sbuf_tensor("xT_s", [D, BSH], f16)
    M_s = nc.alloc_sbuf_tensor("M_s", [D, T * D], f16)
    stg = [nc.alloc_sbuf_tensor(f"stg{i}", [128, NBLK * BW], f16)
           for i in range(NTILES)]
    psum = nc.alloc_psum_tensor("acc", [128, 8 * 512], mybir.dt.float32)

    # DMA-fed sems FIRST and dma_reset restricted to them: the reset's
    # drain sweeps its sem range over ~10us and DMA-completion delivery
    # on a sem is held until the sweep passes it (measured: load sems
    # released in allocation order, ~2-3us apart, starting ~11us).
    s_lds = nc.alloc_semaphore("s_lds")    # sync-ring loads (wait 80 = all 5)
    s_lda = nc.alloc_semaphore("s_lda")    # scalar-ring loads (wait 80 = all 5)
    s_out = nc.alloc_semaphore("s_out")    # all out DMAs (cumulative)
    s_dv = nc.alloc_semaphore("s_dv")      # Vector drains
    s_da = nc.alloc_semaphore("s_da")      # Scalar drains
    s_mm = nc.alloc_semaphore("s_mm")
    s_boot = nc.alloc_semaphore("s_boot")

    all_sems = [s_lds, s_lda, s_out, s_dv, s_da, s_mm, s_boot]
    nums = sorted(s.num for s in all_sems)
    assert nums == list(range(nums[0], nums[-1] + 1)), "sems not contiguous"
    assert [s_lds.num, s_lda.num, s_out.num] == nums[:3]
    sem_range = range(nums[0], nums[-1] + 1)

    nc.gpsimd.dma_reset(range(s_lds.num, s_out.num + 1))

    # drain engine for pair q: even -> Vector, odd -> Scalar
    def dr_sem(q):
        return s_dv if q % 2 == 0 else s_da

    def dr_val(i, q):
        return 4 * i + q // 2 + 1  # per-engine drain count after pair (i, q)

    PW = 2 * BW   # pair width in fp16 cols (1024)
    QT = 2 * PW   # quarter width (2048 fp16 cols, 512KB)
    CW = 2 * BW   # M chunk width (1024 cols)

    NOUT = NPAIR + (NTILES - 1) * 4  # tile0 per-pair + quarters after

    with nc.Block() as block:
        @block.sync
        def _(sync):
            sync.sem_clear(sem_range)
            sync.nop().then_inc(s_boot, 1)
            # critical-path loads (one shared sem; tile 0 waits all 5)
            sync.dma_start(out=xT_s[:, 0:128],
                           in_=xT_d[:, 0:128]).then_inc(s_lds, 16)
            for c in range(NSYNC_CHUNK):
                sync.dma_start(out=M_s[:, c * CW:(c + 1) * CW],
                               in_=M_d[:, c * CW:(c + 1) * CW]
                               ).then_inc(s_lds, 16)
            # tile 0: per-pair outs (fast ramp)
            for q in range(NPAIR):
                sync.wait_ge(dr_sem(q), dr_val(0, q))
                sync.dma_start(
                    out=out_d[0:128, q * PW:(q + 1) * PW],
                    in_=stg[0][:, q * PW:(q + 1) * PW],
                ).then_inc(s_out, 16)
            # tiles 1..7: 512KB quarter outs (pairs 2h, 2h+1)
            for i in range(1, NTILES):
                for h in range(4):
                    sync.wait_ge(s_dv, dr_val(i, 2 * h))
                    sync.wait_ge(s_da, dr_val(i, 2 * h + 1))
                    sync.dma_start(
                        out=out_d[i * 128:(i + 1) * 128, h * QT:(h + 1) * QT],
                        in_=stg[i][:, h * QT:(h + 1) * QT],
                    ).then_inc(s_out, 16)
            sync.wait_ge(s_out, 16 * NOUT)

        @block.tensor
        def _(tensor):
            tensor.wait_ge(s_boot, 1)
            for i in range(NTILES):
                for b in range(NBLK):
                    q = b // 2                      # pair in tile
                    P = i * NPAIR + q               # global pair
                    if i == 0:
                        if b == 0:
                            tensor.wait_ge(s_lds, 80)    # x0 + chunks 0..3
                        elif b == 8:
                            tensor.wait_ge(s_lda, 80)    # xr + chunks 4..7
                    if b % 2 == 0 and P >= 4:       # pair slot reused: drain done?
                        i_, q_ = divmod(P - 4, NPAIR)
                        tensor.wait_ge(dr_sem(q_), dr_val(i_, q_))
                    pb = (P % 4) * 1024 + (b % 2) * 512
                    tensor.matmul(psum[:, pb:pb + 512],
                                  xT_s[:, i * 128:(i + 1) * 128],
                                  M_s[:, b * BW:(b + 1) * BW],
                                  start=True, stop=True).then_inc(s_mm, 1)

        def drain_stream(eng, parity):
            eng.wait_ge(s_boot, 1)
            if parity == 1:
                # scalar ring: the non-critical input loads
                eng.dma_start(out=xT_s[:, 128:BSH],
                              in_=xT_d[:, 128:BSH]).then_inc(s_lda, 16)
                for c in range(NSYNC_CHUNK, NCHUNK):
                    eng.dma_start(out=M_s[:, c * CW:(c + 1) * CW],
                                  in_=M_d[:, c * CW:(c + 1) * CW]
                                  ).then_inc(s_lda, 16)
            for i in range(NTILES):
                for q in range(parity, NPAIR, 2):
                    P = i * NPAIR + q
                    eng.wait_ge(s_mm, i * NBLK + 2 * (q + 1))
                    sem = s_dv if parity == 0 else s_da
                    if parity == 0:
                        eng.tensor_copy(
                            out=stg[i][:, q * PW:(q + 1) * PW],
                            in_=psum[:, (P % 4) * 1024:(P % 4) * 1024 + 1024],
                        ).then_inc(sem, 1)
                    else:
                        eng.copy(
                            out=stg[i][:, q * PW:(q + 1) * PW],
                            in_=psum[:, (P % 4) * 1024:(P % 4) * 1024 + 1024],
                        ).then_inc(sem, 1)

        @block.vector
        def _(vector):
            drain_stream(vector, 0)

        @block.scalar
        def _(scalar):
            drain_stream(scalar, 1)

    return nc


def _prep_inputs(x: np.ndarray, Mcat64: np.ndarray):
    Mb = np.ascontiguousarray(Mcat64.astype(np.float16))
    maps = []
    for c in range(NCORES):
        xT = np.ascontiguousarray(x[c * BSH:(c + 1) * BSH].T.astype(np.float16))
        maps.append({"xT": xT, "M": Mb})
    return maps


def run_on_device(x: np.ndarray, Mcat64: np.ndarray, trace: bool = False):
    from concourse.bass_utils import run_bass_kernel_spmd

    if "nc" not in _CACHE:
        _CACHE["nc"] = _build_nc()
    nc = _CACHE["nc"]

    in_maps = _prep_inputs(x, Mcat64)
    res = run_bass_kernel_spmd(nc, in_maps, core_ids=list(range(NCORES)), trace=trace)
    out = np.empty((B, T, D), dtype=np.float32)
    for c in range(NCORES):
        out[c * BSH:(c + 1) * BSH] = (
            res.results[c]["out"].astype(np.float32).reshape(BSH, T, D))
    return out, res


def kernel(x, W, T):
    x = np.asarray(x, dtype=np.float32)
    W = np.asarray(W, dtype=np.float32)
    assert int(T) == 64 and x.shape == (B, D) and W.shape == (D, D)
    Mcat64 = _expm_table(W)
    out, _ = run_on_device(x, Mcat64, trace=False)
    return out
